# revision 39
# baseline (speedup 1.0000x reference)
"""Trainium2 Bass kernel for the cross-attention fusion module (nn_CAF), v2.

Strategy (8 NeuronCores, sequence-parallel):
  - Each core owns 800 query tokens. It computes softmax rows for its
    queries against all keys, accumulates a partial V @ A product, and the
    partials are summed with an on-chip ReduceScatter so core r ends up
    with output tokens [800r, 800r+800).
  - Host->device traffic is minimized (the axon tunnel is ~60-200MB/s):
    ONE packed fp16 input tensor per core holding the core's token slice,
    6 halo image rows for the depthwise conv, per-core masks, and a 1/8
    shard of the shared weight block. The weight block and the k tensor
    (computed per-slice, incl. pos-enc + bias) are AllGathered on-chip.
    Output is ONE packed fp16 [128, 4800] tensor per core.
  - The compiled XLA/NEFF executable is built once and cached; per call we
    only pack, dispatch, and unpack.
"""
import sys
sys.path.insert(0, '/opt/trn_rl_repo')
import os
import numpy as np

import concourse.bass as bass
import concourse.bacc as bacc
import concourse.tile as tile
from concourse import mybir

F32 = mybir.dt.float32
BF16 = mybir.dt.float16  # fp16
NP_F16 = np.float16

C = 256
RED = 32
H = W = 80
HW = H * W              # 6400
SCALE = RED ** -0.5
N_CORES = 8
SLAB = HW // N_CORES    # 800
ROWS = SLAB // W        # 10
EXP_BIAS = -3.0

AF = mybir.ActivationFunctionType
ALU = mybir.AluOpType

N_CHUNKS = [(i * 128, min(128, SLAB - i * 128)) for i in range((SLAB + 127) // 128)]
CHUNK_PAIRS = [(0, 1), (2, 3), (4, 5), (6,)]

INT8_OUT = True                       # ship outputs as int8 + per-row scales

# ---------------- packed input layout (all fp16) ----------------
# xin1: the big image slice, uploaded while the host packs xin2
XS_W = 4 * SLAB                       # core's tokens [img, cc, 800]
# xin2: per-core small data + W shard
AC2, AC_W = 0, ROWS                   # acore [128, 10]
H62 = 10                              # halo-row-valid mask, [1,512] row-major
OHP2 = 14                             # one-hot of core r-1 (8 cols)
OHN2 = 22                             # one-hot of core r+1 (8 cols)
WSH2 = 30                             # this core's shard of the W block
# --- shared W block (gathered on-chip), columns within [0, WCOL) ---
PEF_O, PEF_W = 0, 352                 # peflat [32,1408] as 4 row-blocks
VW_O, VW_W = 352, 512                 # v_w.T   [128, 2, 256]
PW_O, PW_W = 864, 1024                # proj.T  [128, 4, 256]
GW_O, GW_W = 1888, 1024               # gate.T  [128, 4, 256]
KW_O, KW_W = 2912, 64                 # k_w.T   [128, 2, 32]
QW_O, QW_W = 2976, 64                 # q_w.T   [128, 2, 32]
ABM_O, ABM_W = 3040, 80               # resize A.T tiled 4x [128, 80]
I32_O, I32_W = 3120, 32               # identity [32, 32]
DWW_O, DWW_W = 3152, 196              # dw weights [128, 4, 49]
PEB_O, PEB_W = 3348, 4                # dw bias    [128, 4]
KB_O, QB_O, VB_O, PB_O, GB_O = 3352, 3353, 3354, 3356, 3360
WCOL = 3368
WS = WCOL // N_CORES                  # 421
SCOL = WSH2 + WS                      # 451
OCOL = 3 * 2 * SLAB                   # 4800: [out|rgb|chm] x [cc] x [m]


def _mt(n, width=512):
    return [(i * width, min(width, n - i * width)) for i in range((n + width - 1) // width)]


def build_module():
    nc = bacc.Bacc('TRN2', target_bir_lowering=False, debug=False,
                   num_devices=N_CORES)

    I8 = mybir.dt.int8
    xin1 = nc.dram_tensor('xin1', [128, XS_W], BF16, kind='ExternalInput').ap()
    xin2 = nc.dram_tensor('xin2', [128, SCOL], BF16, kind='ExternalInput').ap()
    if INT8_OUT:
        # +32 int8 cols hold the 8 f32 per-row scales, bitcast to bytes
        oall = nc.dram_tensor('oall', [128, OCOL + 32], I8,
                              kind='ExternalOutput').ap()
    else:
        oall = nc.dram_tensor('oall', [128, OCOL], BF16, kind='ExternalOutput').ap()

    DBG = bool(os.environ.get('KERNEL_DEBUG'))
    if DBG:
        d_peq = nc.dram_tensor('d_peq', [32, SLAB], BF16, kind='ExternalOutput').ap()
        d_krep = nc.dram_tensor('d_krep', [128, 2, HW], BF16, kind='ExternalOutput').ap()
        d_qrep = nc.dram_tensor('d_qrep', [128, 2, SLAB], BF16, kind='ExternalOutput').ap()
        d_vt = nc.dram_tensor('d_vt', [128, 2, 7, C], BF16, kind='ExternalOutput').ap()
        d_vpad = nc.dram_tensor('d_vpad', [4, 128, 16 * 88], BF16, kind='ExternalOutput').ap()
        d_pp = nc.dram_tensor('d_pp', [4, 128, SLAB], BF16, kind='ExternalOutput').ap()
        d_rs = nc.dram_tensor('d_rs', [2, C, SLAB], F32, kind='ExternalOutput').ap()
        d_w = nc.dram_tensor('d_w', [128, WCOL], BF16, kind='ExternalOutput').ap()
        d_h6 = nc.dram_tensor('d_h6', [1, 512], BF16, kind='ExternalOutput').ap()

    groups = [list(range(N_CORES))]

    with tile.TileContext(nc) as tc:
        with (
            tc.tile_pool(name='persist', bufs=1) as PP,
            tc.tile_pool(name='dram', bufs=1, space='DRAM') as DR,
        ):
            k_rep = PP.tile([128, 2, HW], BF16, tag='k_rep')
            q_rep = PP.tile([128, 2, SLAB], BF16, tag='q_rep')
            vt_sb = PP.tile([128, 2, 7, C], BF16, tag='vt_sb')
            vpad = [PP.tile([128, 16 * 88], BF16, tag=f'vpad{i}', name=f'vpad{i}') for i in range(4)]
            vpodd = [PP.tile([128, 16 * 88], BF16, tag=f'vpodd{i}', name=f'vpodd{i}') for i in range(4)]
            pp_t = [PP.tile([128, SLAB], BF16, tag=f'pp{i}', name=f'pp{i}') for i in range(4)]
            dww_f = PP.tile([128, DWW_W], F32, tag='dww')
            dww_sb = dww_f[:].rearrange('p (t k) -> p t k', t=4)
            peb_sb = PP.tile([128, 4], F32, tag='peb')
            ones = PP.tile([1, HW], BF16, tag='ones')
            ebias = PP.tile([128, 1], F32, tag='ebias')
            peq_bf = PP.tile([32, SLAB], BF16, tag='peq_bf')

            wc_dr = DR.tile([128, WS], BF16, name='wc_dr')
            wall = DR.tile([N_CORES, 128, WS], BF16, name='wall')
            estrip = DR.tile([128, 2, 2, 480], BF16, name='estrip')
            gstr = DR.tile([N_CORES, 128, 2, 2, 480], BF16, name='gstr')
            kc_dr = DR.tile([32, 2, SLAB], BF16, name='kc_dr')
            kall = DR.tile([N_CORES, 32, 2, SLAB], BF16, name='kall')
            partial = [DR.tile([N_CORES, C, SLAB], F32, name=f'partial{i}') for i in range(2)]
            rs_out = [DR.tile([C, SLAB], F32, name=f'rs_out{i}') for i in range(2)]

            def w_unpack(dst, off, width, p0=0, p1=128):
                # dst: flat-col AP view [p1-p0, width]; src: W cols [off, off+width)
                b = off + width
                for r in range(off // WS, (b - 1) // WS + 1):
                    s0, s1 = max(off, r * WS), min(b, (r + 1) * WS)
                    nc.sync.dma_start(dst[:, s0 - off:s1 - off],
                                      wall[r, p0:p1, s0 - r * WS:s1 - r * WS])

            nc.vector.memset(ones[:], 1.0)
            nc.vector.memset(ebias[:], EXP_BIAS)

            # ---------------- phase A: convs + pos-enc resize ----------------
            with (
                tc.tile_pool(name='phA', bufs=1) as PA,
                tc.tile_pool(name='evA', bufs=4) as EA,
                tc.tile_pool(name='psA', bufs=2, space='PSUM') as PSA,
            ):
                xs_sb = PA.tile([128, XS_W], BF16, tag='xs_sb')
                s2_sb = PA.tile([128, SCOL], BF16, tag='s2_sb')
                # ---- load packed inputs; gather the shared W block ----
                nc.sync.dma_start(s2_sb[:], xin2[:])
                nc.sync.dma_start(xs_sb[:], xin1[:])
                nc.sync.dma_start(wc_dr[:], s2_sb[:, WSH2:WSH2 + WS])
                nc.gpsimd.collective_compute(
                    'AllGather', ALU.bypass, replica_groups=groups,
                    ins=[wc_dr[:].opt()], outs=[wall[:].opt()])
                dww_h = PA.tile([128, DWW_W + PEB_W], BF16, tag='dww_h')
                w_unpack(dww_h[:, 0:DWW_W], DWW_O, DWW_W)
                w_unpack(dww_h[:, DWW_W:], PEB_O, PEB_W)
                nc.vector.tensor_copy(dww_f[:], dww_h[:, 0:DWW_W])
                nc.vector.tensor_copy(peb_sb[:], dww_h[:, DWW_W:])

                xs_v = xs_sb[:].rearrange('p (i c m) -> p i c m', i=2, c=2)
                ac_v = s2_sb[:, AC2:AC2 + AC_W]

                # edge strips (first/last 3 image rows of the slice) -> AllGather
                for img in range(2):
                    for cc in range(2):
                        nc.sync.dma_start(estrip[:, img, cc, 0:240],
                                          xs_v[:, img, cc, 0:240])
                        nc.sync.dma_start(estrip[:, img, cc, 240:480],
                                          xs_v[:, img, cc, 560:800])
                nc.gpsimd.collective_compute(
                    'AllGather', ALU.bypass, replica_groups=groups,
                    ins=[estrip[:].opt()], outs=[gstr[:].opt()])
                gst_sb = PA.tile([128, N_CORES, 2, 2, 480], BF16, tag='gst')
                for s in range(N_CORES):
                    nc.sync.dma_start(gst_sb[:, s, :, :, :], gstr[s, :, :, :, :])
                # one-hot select: above rows = core r-1's bottom strip, below
                # rows = core r+1's top strip (zero at the image boundary).
                hal = PA.tile([128, 2, 2, 480], BF16, tag='hal')
                oh_f = PA.tile([128, 16], F32, tag='oh_f')
                nc.vector.tensor_copy(oh_f[:], s2_sb[:, OHP2:OHP2 + 16])
                for s in range(N_CORES):
                    ohp = oh_f[:, s:s + 1]
                    ohn = oh_f[:, 8 + s:8 + s + 1]
                    if s == 0:
                        nc.vector.tensor_scalar_mul(
                            hal[:, :, :, 0:240], gst_sb[:, s, :, :, 240:480], ohp)
                        nc.vector.tensor_scalar_mul(
                            hal[:, :, :, 240:480], gst_sb[:, s, :, :, 0:240], ohn)
                    else:
                        nc.vector.scalar_tensor_tensor(
                            hal[:, :, :, 0:240], gst_sb[:, s, :, :, 240:480], ohp,
                            hal[:, :, :, 0:240], op0=ALU.mult, op1=ALU.add)
                        nc.vector.scalar_tensor_tensor(
                            hal[:, :, :, 240:480], gst_sb[:, s, :, :, 0:240], ohn,
                            hal[:, :, :, 240:480], op0=ALU.mult, op1=ALU.add)
                xh_v = hal[:]

                pef_sb = PA.tile([32, 11 * 128], BF16, tag='pef')
                kw_f = PA.tile([128, KW_W], BF16, tag='kw')
                qw_f = PA.tile([128, QW_W], BF16, tag='qw')
                vw_f = PA.tile([128, VW_W], BF16, tag='vw')
                ab_sb = PA.tile([128, ABM_W], BF16, tag='ab')
                i32_sb = PA.tile([32, I32_W], BF16, tag='i32')
                kb_sb = PA.tile([1, RED], BF16, tag='kb')
                qb_sb = PA.tile([1, RED], BF16, tag='qb')
                vb_sb = PA.tile([1, C], BF16, tag='vb')
                h6_sb = PA.tile([1, 512], BF16, tag='h6')
                k_sl = PA.tile([32, 2, SLAB], BF16, tag='k_sl')

                for k4 in range(4):
                    nc.sync.dma_start(pef_sb[0:32, 352 * k4:352 * (k4 + 1)],
                                      wall[0, 32 * k4:32 * (k4 + 1), PEF_O:PEF_O + 352])
                w_unpack(kw_f[:], KW_O, KW_W)
                w_unpack(qw_f[:], QW_O, QW_W)
                w_unpack(vw_f[:], VW_O, VW_W)
                w_unpack(ab_sb[:], ABM_O, ABM_W)
                w_unpack(i32_sb[:], I32_O, I32_W, p0=0, p1=32)
                nc.sync.dma_start(kb_sb[:], wall[KB_O // WS, 0:32, KB_O % WS:KB_O % WS + 1])
                nc.sync.dma_start(qb_sb[:], wall[QB_O // WS, 0:32, QB_O % WS:QB_O % WS + 1])
                nc.sync.dma_start(vb_sb[:], wall[VB_O // WS, :, VB_O % WS:VB_O % WS + 2])
                nc.sync.dma_start(h6_sb[:], s2_sb[:, H62:H62 + 4])
                kw_v = kw_f[:].rearrange('p (c r) -> p c r', c=2)
                qw_v = qw_f[:].rearrange('p (c r) -> p c r', c=2)
                vw_v = vw_f[:].rearrange('p (c r) -> p c r', c=2)

                # --- pos-enc bilinear resize, stage 1 (contraction over i) ---
                t1f = PA.tile([128, 11, 80], BF16, tag='t1f')
                for ch in range(11):
                    ps = PSA.tile([128, 80], F32, tag='pa')
                    nc.tensor.matmul(ps[:], pef_sb[:, 128 * ch:128 * (ch + 1)],
                                     ab_sb[0:32, :], start=True, stop=True)
                    nc.vector.tensor_copy(t1f[:, ch, :], ps[:])
                # --- stage 2, core's rows only: peq[r, row*80+x] ---
                for r in range(RED):
                    ch, sub = r // 3, (r % 3) * 32
                    ps2 = PSA.tile([ROWS, 80], F32, tag='pa')
                    nc.tensor.matmul(ps2[:], ac_v[sub:sub + 32, :],
                                     t1f[sub:sub + 32, ch, :], start=True, stop=True)
                    ev2 = EA.tile([ROWS, 80], BF16, tag='pe2se')
                    nc.scalar.copy(ev2[:], ps2[:])
                    nc.sync.dma_start(peq_bf[r:r + 1, :], ev2[:])

                # --- k + q convs on the core's slice, + pe + bias ---
                for img in range(2):
                    for wv, bv, dst in ((kw_v, kb_sb, None), (qw_v, qb_sb, q_rep)):
                        ps = PSA.tile([32, SLAB], F32, tag='pa')
                        for m0, mw in _mt(SLAB):
                            nc.tensor.matmul(ps[:, m0:m0 + mw], wv[:, 0, :],
                                             xs_v[:, img, 0, m0:m0 + mw],
                                             start=True, stop=False)
                            nc.tensor.matmul(ps[:, m0:m0 + mw], wv[:, 1, :],
                                             xs_v[:, img, 1, m0:m0 + mw],
                                             start=False, stop=False)
                            nc.tensor.matmul(ps[:, m0:m0 + mw], bv[:],
                                             ones[:, m0:m0 + mw],
                                             start=False, stop=False)
                            nc.tensor.matmul(ps[:, m0:m0 + mw], i32_sb[:],
                                             peq_bf[:, m0:m0 + mw],
                                             start=False, stop=True)
                        if dst is None:
                            nc.scalar.copy(k_sl[0:32, img, :], ps[:])
                        else:
                            nc.scalar.copy(dst[0:32, img, :], ps[:])
                            nc.sync.dma_start(dst[32:64, img, :], dst[0:32, img, :])

                # --- gather k across cores -> full k_rep ---
                nc.sync.dma_start(kc_dr[:], k_sl[:])
                nc.gpsimd.collective_compute(
                    'AllGather', ALU.bypass, replica_groups=groups,
                    ins=[kc_dr[:].opt()], outs=[kall[:].opt()])
                for img in range(2):
                    for s in range(N_CORES):
                        nc.sync.dma_start(k_rep[0:32, img, SLAB * s:SLAB * (s + 1)],
                                          kall[s, :, img, :])
                    nc.sync.dma_start(k_rep[32:64, img, :], k_rep[0:32, img, :])

                # --- VT conv: vt[n, c] for core's n slice ---
                for img in range(2):
                    for ci, (n0, nw) in enumerate(N_CHUNKS):
                        ps = PSA.tile([128, C], F32, tag='pa')
                        nc.tensor.matmul(ps[:nw, :], xs_v[:, img, 0, n0:n0 + nw],
                                         vw_v[:, 0, :], start=True, stop=False)
                        nc.tensor.matmul(ps[:nw, :], xs_v[:, img, 1, n0:n0 + nw],
                                         vw_v[:, 1, :], start=False, stop=False)
                        nc.tensor.matmul(ps[:nw, :], ones[0:1, n0:n0 + nw],
                                         vb_sb[:], start=False, stop=True)
                        nc.vector.tensor_copy(vt_sb[:nw, img, ci, :], ps[:nw, :])

                # --- v conv on own rows + halo rows -> padded dwconv input ---
                for img in range(2):
                    for cc in range(2):
                        ps10 = PSA.tile([128, SLAB], F32, tag='pa')
                        for m0, mw in _mt(SLAB):
                            nc.tensor.matmul(ps10[:, m0:m0 + mw],
                                             vw_v[:, 0, 128 * cc:128 * (cc + 1)],
                                             xs_v[:, img, 0, m0:m0 + mw],
                                             start=True, stop=False)
                            nc.tensor.matmul(ps10[:, m0:m0 + mw],
                                             vw_v[:, 1, 128 * cc:128 * (cc + 1)],
                                             xs_v[:, img, 1, m0:m0 + mw],
                                             start=False, stop=False)
                            nc.tensor.matmul(ps10[:, m0:m0 + mw],
                                             vb_sb[:, 128 * cc:128 * (cc + 1)],
                                             ones[:, m0:m0 + mw],
                                             start=False, stop=True)
                        ps6 = PSA.tile([128, 480], F32, tag='pa6')
                        nc.tensor.matmul(ps6[:], vw_v[:, 0, 128 * cc:128 * (cc + 1)],
                                         xh_v[:, img, 0, :], start=True, stop=False)
                        nc.tensor.matmul(ps6[:], vw_v[:, 1, 128 * cc:128 * (cc + 1)],
                                         xh_v[:, img, 1, :], start=False, stop=False)
                        nc.tensor.matmul(ps6[:], vb_sb[:, 128 * cc:128 * (cc + 1)],
                                         h6_sb[:, 0:480], start=False, stop=True)
                        vp = vpad[img * 2 + cc]
                        nc.vector.memset(vp[:], 0.0)
                        vp3 = vp[:].rearrange('p (r x) -> p r x', r=16)
                        nc.vector.tensor_copy(
                            vp3[:, 0:3, 3:83],
                            ps6[:, 0:240].rearrange('p (r x) -> p r x', r=3))
                        nc.vector.tensor_copy(
                            vp3[:, 3:13, 3:83],
                            ps10[:].rearrange('p (r x) -> p r x', r=ROWS))
                        nc.vector.tensor_copy(
                            vp3[:, 13:16, 3:83],
                            ps6[:, 240:480].rearrange('p (r x) -> p r x', r=3))
                        vo = vpodd[img * 2 + cc]
                        nc.vector.tensor_copy(vo[:, 0:1407], vp[:, 1:1408])
                        nc.vector.memset(vo[:, 1407:1408], 0.0)

            # ------------- dwconv emission helper (interleaved later) -------------
            dw_items = []
            for t in range(4):
                for dy in range(7):
                    for dx in range(7):
                        dw_items.append((t, dy, dx))

            def emit_dw(n):
                for _ in range(n):
                    if not dw_items:
                        return
                    t, dy, dx = dw_items.pop(0)
                    par = dx % 2
                    base = vpodd[t] if par else vpad[t]
                    c0 = dx - par
                    src = base[:].rearrange('p (r x) -> p r x', x=88)[:, dy:dy + ROWS, c0:c0 + 80]
                    dst = pp_t[t][:].rearrange('p (r x) -> p r x', x=80)
                    wap = dww_sb[:, t, dy * 7 + dx:dy * 7 + dx + 1]
                    if dy == 0 and dx == 0:
                        nc.vector.tensor_scalar_mul(dst[:], src, wap)
                    else:
                        nc.vector.scalar_tensor_tensor(
                            dst[:], src, wap, dst[:], op0=ALU.mult, op1=ALU.add)

            # ---------------- attention phases ----------------
            with (
                tc.tile_pool(name='attn', bufs=1) as AT,
                tc.tile_pool(name='evT', bufs=4) as ET,
                tc.tile_pool(name='psS', bufs=1, space='PSUM') as PSS,
                tc.tile_pool(name='psV', bufs=3, space='PSUM') as PSV,
            ):
                HM = HW // 2
                racc = [AT.tile([128, 7, 8], F32, tag=f'racc{a}',
                                name=f'racc{a}') for a in range(2)]
                rsum = [AT.tile([128, 7], F32, tag=f'rsum{a}', name=f'rsum{a}') for a in range(2)]
                rinv = [AT.tile([128, 7], F32, tag=f'rinv{a}', name=f'rinv{a}') for a in range(2)]
                vtp = [AT.tile([128, 7, C], BF16, tag=f'vtp{a}', name=f'vtp{a}') for a in range(2)]
                H_MACROS = [(0, 1024), (1024, 1024), (2048, 1024), (3072, 128)]

                def s_exp_half(a, h, e_h):
                    qi, ki = (0, 1) if a == 0 else (1, 0)
                    for pair in CHUNK_PAIRS:
                        for mi, (m0, mw) in enumerate(H_MACROS):
                            pss = []
                            for g, ci in enumerate(pair):
                                n0, nw = N_CHUNKS[ci]
                                ps = PSS.tile([128, 1024], F32, tag='s')
                                pss.append((ps, ci, nw))
                                for sm0, smw in _mt(mw):
                                    km = h * HM + m0 + sm0
                                    nc.tensor.matmul(
                                        ps[:nw, sm0:sm0 + smw],
                                        q_rep[32 * g:32 * g + 32, qi, n0:n0 + nw],
                                        k_rep[32 * g:32 * g + 32, ki, km:km + smw],
                                        start=True, stop=True,
                                        tile_position=(32 * g, 0))
                            for ps, ci, nw in pss:
                                nc.scalar.activation(
                                    e_h[:nw, ci, m0:m0 + mw], ps[:nw, :mw],
                                    AF.Exp, bias=ebias[:nw, 0:1], scale=SCALE,
                                    accum_out=racc[a][:nw, ci, h * 4 + mi:h * 4 + mi + 1])

                def finalize(a):
                    nc.vector.reduce_sum(rsum[a][:], racc[a][:],
                                         axis=mybir.AxisListType.X)
                    nc.vector.reciprocal(rinv[a][:], rsum[a][:])
                    for ci, (n0, nw) in enumerate(N_CHUNKS):
                        nc.vector.tensor_scalar_mul(
                            vtp[a][:nw, ci, :], vt_sb[:nw, a, ci, :],
                            rinv[a][:nw, ci:ci + 1])

                def ve_half(a, h, e_h):
                    slabs = [h * 4 + i for i in range(4)]
                    for gi0 in range(0, 4, 2):
                        grp = slabs[gi0:gi0 + 2]
                        for cc in range(2):
                            pst = []
                            for slab in grp:
                                ps = PSV.tile([128, SLAB], F32, tag='ve')
                                pst.append(ps)
                                lm = (slab - h * 4) * SLAB
                                for ci, (n0, nw) in enumerate(N_CHUNKS):
                                    for off, mw in ((0, 512), (512, 288)):
                                        nc.tensor.matmul(
                                            ps[:, off:off + mw],
                                            vtp[a][:nw, ci, 128 * cc:128 * (cc + 1)],
                                            e_h[:nw, ci, lm + off:lm + off + mw],
                                            start=(ci == 0), stop=(ci == 6))
                            for k, slab in enumerate(grp):
                                ev = ET.tile([128, SLAB], F32, tag='vee')
                                nc.scalar.copy(ev[:], pst[k][:])
                                nc.sync.dma_start(
                                    partial[a][slab, 128 * cc:128 * (cc + 1), :],
                                    ev[:])
                            emit_dw(10)

                def e_tile(nm):
                    return AT.tile([128, 7, HM], BF16, tag='E', bufs=2, name=nm)

                e_a0 = e_tile('e_a0')
                s_exp_half(0, 0, e_a0)
                emit_dw(10)
                e_a1 = e_tile('e_a1')
                s_exp_half(0, 1, e_a1)
                finalize(0)
                emit_dw(10)
                ve_half(0, 0, e_a0)
                e_b0 = e_tile('e_b0')
                s_exp_half(1, 0, e_b0)
                ve_half(0, 1, e_a1)
                nc.gpsimd.collective_compute(
                    'ReduceScatter', ALU.add, replica_groups=groups,
                    ins=[partial[0].opt()], outs=[rs_out[0].opt()])
                e_b1 = e_tile('e_b1')
                s_exp_half(1, 1, e_b1)
                finalize(1)
                ve_half(1, 0, e_b0)
                ve_half(1, 1, e_b1)
                nc.gpsimd.collective_compute(
                    'ReduceScatter', ALU.add, replica_groups=groups,
                    ins=[partial[1].opt()], outs=[rs_out[1].opt()])
                emit_dw(200)
                if DBG:
                    nc.sync.dma_start(d_peq[:], peq_bf[:])
                    nc.sync.dma_start(d_krep[:], k_rep[:])
                    nc.sync.dma_start(d_qrep[:], q_rep[:])
                    nc.sync.dma_start(d_vt[:], vt_sb[:])
                    nc.sync.dma_start(d_h6[:], h6_sb[:])
                    for a in range(2):
                        nc.sync.dma_start(d_rs[a, :, :], rs_out[a][:])
                    for t in range(4):
                        nc.sync.dma_start(d_pp[t, :, :], pp_t[t][:])
                        nc.sync.dma_start(d_vpad[t, :, :], vpad[t][:])
                    for r in range(N_CORES):
                        wt = AT.tile([128, WS], BF16, tag='wtmp', bufs=2,
                                     name=f'wt{r}')
                        nc.sync.dma_start(wt[:], wall[r, :, :])
                        nc.sync.dma_start(d_w[:, WS * r:WS * (r + 1)], wt[:])

            # ---------------- phase D: dw-bias + proj + gate + blend ----------------
            with (
                tc.tile_pool(name='phD', bufs=1) as PD,
                tc.tile_pool(name='evD', bufs=4) as ED,
                tc.tile_pool(name='psD', bufs=2, space='PSUM') as PSD,
            ):
                pw_f = PD.tile([128, PW_W], BF16, tag='pw')
                gw_f = PD.tile([128, GW_W], BF16, tag='gw')
                pb_sb = PD.tile([1, 2 * C], BF16, tag='pb')
                gb_sb = PD.tile([1, C], BF16, tag='gb')
                w_unpack(pw_f[:], PW_O, PW_W)
                w_unpack(gw_f[:], GW_O, GW_W)
                nc.sync.dma_start(pb_sb[:], wall[PB_O // WS, :, PB_O % WS:PB_O % WS + 4])
                nc.sync.dma_start(gb_sb[:], wall[GB_O // WS, :, GB_O % WS:GB_O % WS + 2])
                pw_sb = pw_f[:].rearrange('p (t c) -> p t c', t=4)
                gw_sb = gw_f[:].rearrange('p (t c) -> p t c', t=4)
                pb_v = pb_sb[:].rearrange('p (i c) -> p i c', i=2)

                asum = PD.tile([128, 2, 2, SLAB], F32, tag='asum')
                for a in range(2):
                    for cc in range(2):
                        nc.sync.dma_start(asum[:, a, cc, :],
                                          rs_out[a][128 * cc:128 * (cc + 1), :])
                pi = PD.tile([128, 2, 2, SLAB], BF16, tag='pi')
                for img in range(2):
                    for cc in range(2):
                        t = img * 2 + cc
                        nc.vector.scalar_tensor_tensor(
                            pi[:, img, cc, :], pp_t[t][:], peb_sb[:, t:t + 1],
                            asum[:, img, cc, :], op0=ALU.add, op1=ALU.add)
                attf = PD.tile([128, 2, 2, SLAB], F32, tag='attf')
                gi_t = PD.tile([128, 4, SLAB], BF16, tag='gi')
                for img in range(2):
                    for mc in range(2):
                        ps = PSD.tile([128, SLAB], F32, tag='proj')
                        for m0, mw in _mt(SLAB):
                            nc.tensor.matmul(ps[:, m0:m0 + mw],
                                             pw_sb[:, img * 2, 128 * mc:128 * (mc + 1)],
                                             pi[:, img, 0, m0:m0 + mw],
                                             start=True, stop=False)
                            nc.tensor.matmul(ps[:, m0:m0 + mw],
                                             pw_sb[:, img * 2 + 1, 128 * mc:128 * (mc + 1)],
                                             pi[:, img, 1, m0:m0 + mw],
                                             start=False, stop=False)
                            nc.tensor.matmul(ps[:, m0:m0 + mw],
                                             pb_v[:, img, 128 * mc:128 * (mc + 1)],
                                             ones[:, m0:m0 + mw],
                                             start=False, stop=True)
                        nc.vector.tensor_copy(attf[:, img, mc, :], ps[:])
                        nc.scalar.copy(gi_t[:, img * 2 + mc, :], ps[:])
                gsig = PD.tile([128, 2, SLAB], BF16, tag='gsig')
                for mc in range(2):
                    ps = PSD.tile([128, SLAB], F32, tag='gate')
                    for m0, mw in _mt(SLAB):
                        for kc in range(4):
                            nc.tensor.matmul(ps[:, m0:m0 + mw],
                                             gw_sb[:, kc, 128 * mc:128 * (mc + 1)],
                                             gi_t[:, kc, m0:m0 + mw],
                                             start=(kc == 0), stop=False)
                        nc.tensor.matmul(ps[:, m0:m0 + mw],
                                         gb_sb[:, 128 * mc:128 * (mc + 1)],
                                         ones[:, m0:m0 + mw],
                                         start=False, stop=True)
                    nc.scalar.activation(gsig[:, mc, :], ps[:], AF.Sigmoid)
                # blend: out = chm + g*(rgb - chm); pack [out|rgb|chm]
                if INT8_OUT:
                    osc_sb = PD.tile([128, 8], F32, tag='osc')
                    nc.vector.memset(osc_sb[:], 0.0)
                for mc in range(2):
                    d = ED.tile([128, SLAB], F32, tag='d')
                    nc.vector.tensor_sub(d[:], attf[:, 0, mc, :], attf[:, 1, mc, :])
                    nc.vector.tensor_mul(d[:], d[:], gsig[:, mc, :])
                    do = ED.tile([128, SLAB], F32, tag='dof')
                    nc.vector.tensor_add(do[:], d[:], attf[:, 1, mc, :])
                    srcs = [do[:], attf[:, 0, mc, :], attf[:, 1, mc, :]]
                    for oi, src in enumerate(srcs):
                        col = oi * 2 * SLAB + mc * SLAB
                        if INT8_OUT:
                            rmax = ED.tile([128, 1], F32, tag='rmax')
                            nc.vector.reduce_max(rmax[:], src,
                                                 axis=mybir.AxisListType.X,
                                                 apply_absolute_value=True)
                            nc.vector.tensor_scalar_max(rmax[:], rmax[:], 1e-30)
                            nc.scalar.copy(
                                osc_sb[:, oi * 2 + mc:oi * 2 + mc + 1], rmax[:])
                            sinv = ED.tile([128, 1], F32, tag='sinv')
                            nc.vector.reciprocal(sinv[:], rmax[:])
                            nc.vector.tensor_scalar_mul(sinv[:], sinv[:], 126.5)
                            q = ED.tile([128, SLAB], I8, tag='q')
                            nc.vector.tensor_scalar_mul(q[:], src, sinv[:, 0:1])
                            nc.sync.dma_start(oall[:, col:col + SLAB], q[:])
                        else:
                            o16 = ED.tile([128, SLAB], BF16, tag='do')
                            nc.vector.tensor_copy(o16[:], src)
                            nc.sync.dma_start(oall[:, col:col + SLAB], o16[:])
                if INT8_OUT:
                    # issue from the activation engine: the scale writes are
                    # activation-engine copies, so queue order guarantees they
                    # land before this DMA reads the tile through the bitcast.
                    nc.scalar.dma_start(oall[:, OCOL:OCOL + 32],
                                        osc_sb[:].bitcast(I8))

    nc.compile()
    from concourse.bass_interp import get_hw_module
    nc.m = get_hw_module(nc.m)
    return nc


def build_resize_matrix():
    scale = 32 / 80.0
    A = np.zeros((80, 32), np.float64)
    for y in range(80):
        src = (y + 0.5) * scale - 0.5
        for i in range(32):
            A[y, i] = max(0.0, 1.0 - abs(src - i))
        A[y] /= A[y].sum()
    return A.astype(np.float32)


def _pack_peflat(pos_enc):
    out = np.zeros((32, 11 * 128), np.float32)
    for r in range(RED):
        ch, t = r // 3, r % 3
        out[:, 128 * ch + 32 * t:128 * ch + 32 * t + 32] = pos_enc[0, r].T
    return out


def build_wblock(pos_enc, q_w, q_b, k_w, k_b, v_w, v_b,
                 rgb_pe_w, rgb_pe_b, chm_pe_w, chm_pe_b,
                 rgb_proj_w, rgb_proj_b, chm_proj_w, chm_proj_b,
                 gate_w, gate_b):
    Wb = np.zeros((128, WCOL), NP_F16)
    pef = _pack_peflat(pos_enc)
    for k4 in range(4):
        Wb[32 * k4:32 * (k4 + 1), PEF_O:PEF_O + 352] = pef[:, 352 * k4:352 * (k4 + 1)]
    Wb[:, VW_O:VW_O + VW_W] = v_w.T.reshape(2, 128, C).transpose(1, 0, 2).reshape(128, VW_W)
    Wb[:, PW_O:PW_O + PW_W] = (np.stack([rgb_proj_w.T, chm_proj_w.T])
                               .reshape(4, 128, C).transpose(1, 0, 2).reshape(128, PW_W))
    Wb[:, GW_O:GW_O + GW_W] = gate_w.T.reshape(4, 128, C).transpose(1, 0, 2).reshape(128, GW_W)
    Wb[:, KW_O:KW_O + KW_W] = k_w.T.reshape(2, 128, RED).transpose(1, 0, 2).reshape(128, KW_W)
    Wb[:, QW_O:QW_O + QW_W] = q_w.T.reshape(2, 128, RED).transpose(1, 0, 2).reshape(128, QW_W)
    Wb[:, ABM_O:ABM_O + ABM_W] = np.tile(build_resize_matrix().T, (4, 1))
    Wb[0:32, I32_O:I32_O + I32_W] = np.eye(32, dtype=np.float32)
    Wb[:, DWW_O:DWW_O + DWW_W] = (np.stack([rgb_pe_w.reshape(C, 49), chm_pe_w.reshape(C, 49)])
                                  .reshape(4, 128, 49).transpose(1, 0, 2).reshape(128, DWW_W))
    Wb[:, PEB_O:PEB_O + PEB_W] = np.stack([rgb_pe_b, chm_pe_b]).reshape(4, 128).T
    Wb[0:32, KB_O] = k_b
    Wb[0:32, QB_O] = q_b
    Wb[:, VB_O:VB_O + 2] = v_b.reshape(128, 2)
    Wb[:, PB_O:PB_O + 4] = np.concatenate([rgb_proj_b, chm_proj_b]).reshape(128, 4)
    Wb[:, GB_O:GB_O + 2] = gate_b.reshape(128, 2)
    return Wb


def _pool():
    if 'pool' not in _CACHE:
        from concurrent.futures import ThreadPoolExecutor
        _CACHE['pool'] = ThreadPoolExecutor(max_workers=8)
    return _CACHE['pool']


def pack_x1(rgb, chm):
    x1 = np.empty((N_CORES, 128, XS_W), NP_F16)
    v = x1.reshape(N_CORES, 128, 2, 2, SLAB)

    def job(img, im):
        # [cc*128+p, r*800+j] -> [r, p, cc, j], cast f32->f16 in the assign
        v[:, :, img] = im.reshape(2, 128, N_CORES, SLAB).transpose(2, 1, 0, 3)

    fs = [_pool().submit(job, img, im) for img, im in ((0, rgb), (1, chm))]
    [f.result() for f in fs]
    return x1.reshape(N_CORES * 128, XS_W)


def pack_x2(pos_enc, q_w, q_b, k_w, k_b, v_w, v_b,
            rgb_pe_w, rgb_pe_b, chm_pe_w, chm_pe_b,
            rgb_proj_w, rgb_proj_b, chm_proj_w, chm_proj_b,
            gate_w, gate_b):
    if 'x2_tmpl' not in _CACHE:
        tm = np.zeros((N_CORES, 128, SCOL), NP_F16)
        h6 = np.zeros((N_CORES, 512), NP_F16)
        for r in range(N_CORES):
            for k in range(3):
                if 0 <= r * ROWS - 3 + k < H:
                    h6[r, k * W:(k + 1) * W] = 1.0
                if 0 <= r * ROWS + ROWS + k < H:
                    h6[r, (3 + k) * W:(4 + k) * W] = 1.0
            if r > 0:
                tm[r, :, OHP2 + r - 1] = 1.0
            if r < N_CORES - 1:
                tm[r, :, OHN2 + r + 1] = 1.0
        A = build_resize_matrix()
        for r in range(N_CORES):
            tm[r, :, AC2:AC2 + AC_W] = np.tile(
                A.T[:, r * ROWS:(r + 1) * ROWS], (4, 1))
        tm[:, :, H62:H62 + 4] = h6.reshape(N_CORES, 128, 4)
        _CACHE['x2_tmpl'] = tm
    x2 = _CACHE['x2_tmpl'].copy()
    Wb = build_wblock(pos_enc, q_w, q_b, k_w, k_b, v_w, v_b,
                      rgb_pe_w, rgb_pe_b, chm_pe_w, chm_pe_b,
                      rgb_proj_w, rgb_proj_b, chm_proj_w, chm_proj_b,
                      gate_w, gate_b)
    x2[:, :, WSH2:] = Wb.reshape(128, N_CORES, WS).transpose(1, 0, 2)
    return x2.reshape(N_CORES * 128, SCOL)


def pack_inputs(rgb, chm, **weights):
    return pack_x1(rgb, chm), pack_x2(**weights)


_CACHE = {}


def _build_runner():
    import jax
    from jax.sharding import Mesh, PartitionSpec
    from jax.experimental.shard_map import shard_map
    from concourse.bass2jax import (_bass_exec_p, partition_id_tensor,
                                    install_neuronx_cc_hook)
    install_neuronx_cc_hook()
    nc = build_module()

    partition_name = nc.partition_id_tensor.name if nc.partition_id_tensor else None
    in_names, out_names, out_avals = [], [], []
    for alloc in nc.m.functions[0].allocations:
        if not isinstance(alloc, mybir.MemoryLocationSet):
            continue
        name = alloc.memorylocations[0].name
        if alloc.kind == 'ExternalInput':
            if name != partition_name:
                in_names.append(name)
        elif alloc.kind == 'ExternalOutput':
            out_names.append(name)
            out_avals.append(jax.core.ShapedArray(
                tuple(alloc.tensor_shape), mybir.dt.np(alloc.dtype)))
    in_names_all = list(in_names)
    if partition_name is not None:
        in_names_all.append(partition_name)

    def _body(*args):
        operands = list(args)
        if partition_name is not None:
            operands.append(partition_id_tensor())
        return tuple(_bass_exec_p.bind(
            *operands, out_avals=tuple(out_avals), in_names=tuple(in_names_all),
            out_names=tuple(out_names), lowering_input_output_aliases=(),
            sim_require_finite=True, sim_require_nnan=True, nc=nc))

    devices = jax.devices()[:N_CORES]
    mesh = Mesh(np.asarray(devices), ('core',))
    fn = jax.jit(shard_map(
        _body, mesh=mesh, in_specs=(PartitionSpec('core'),) * len(in_names),
        out_specs=(PartitionSpec('core'),) * len(out_names), check_rep=False))
    sharding = jax.sharding.NamedSharding(mesh, PartitionSpec('core'))
    return {'nc': nc, 'fn': fn, 'in_names': in_names,
            'out_names': out_names, 'sharding': sharding, 'jax': jax}


def get_kernel():
    if 'R' not in _CACHE:
        _CACHE['R'] = _build_runner()
    return _CACHE['R']


def run_packed(x1, x2):
    R = get_kernel()
    outs = R['fn'](x1, x2)
    if len(outs) == 1:
        return {R['out_names'][0]: np.asarray(outs[0])}
    futs = [_pool().submit(np.asarray, o) for o in outs]
    return {n: f.result() for n, f in zip(R['out_names'], futs)}


def unpack_outputs(res):
    if INT8_OUT:
        raw = res['oall']
        a8 = raw[:, :OCOL].reshape(N_CORES, 128, 3, 2, SLAB)
        a8 = a8.transpose(2, 3, 1, 0, 4)            # [o, mc, p, r, j] view
        sc = np.ascontiguousarray(raw[:, OCOL:]).view('<f4')
        sc = sc.reshape(N_CORES, 128, 8)[:, :, :6].reshape(N_CORES, 128, 3, 2)
        sc = sc.transpose(2, 3, 1, 0)[..., None] * (1.0 / 126.5)
        full = np.multiply(a8, sc, dtype=np.float32).reshape(3, C, HW)
    else:
        arr = res['oall'].reshape(N_CORES, 128, 3, 2, SLAB)
        full = arr.transpose(2, 3, 1, 0, 4).reshape(3, C, HW).astype(np.float32)
    return tuple(full[i].reshape(1, C, H, W) for i in range(3))


def _outputs_valid(res):
    if not INT8_OUT:
        return True
    raw = res['oall']
    sc = np.ascontiguousarray(raw[:, OCOL:]).view('<f4')[:, :6]
    if not (np.isfinite(sc).all() and (np.abs(sc) < 1e4).all()):
        return False
    # quant maps each row's absmax to ~126, so a healthy payload has
    # near-full-scale values in every core's slab
    return bool(np.abs(raw[:, :OCOL]).max() >= 64)


def _fetch_dequant(o):
    """Stage async D2H for all 8 output shards at once, then dequantize
    serially as each lands — shard i's host math overlaps shard i+1's
    transfer. Returns (full[3, C, HW] f32, ok)."""
    full = np.empty((3, 2, 128, N_CORES, SLAB), np.float32)
    ok = True
    shards = sorted(o.addressable_shards, key=lambda s: s.index[0].start or 0)
    datas = [sh.data for sh in shards]
    for d in datas:
        try:
            d.copy_to_host_async()
        except Exception:
            pass
    for i, d in enumerate(datas):
        raw = np.asarray(d)                             # [128, OCOL+32] int8
        sc = np.ascontiguousarray(raw[:, OCOL:]).view('<f4')[:, :6]
        if not (np.isfinite(sc).all() and (np.abs(sc) < 1e4).all()
                and np.abs(raw[:, :OCOL]).max() >= 64):
            ok = False
            continue
        a8 = raw[:, :OCOL].reshape(128, 3, 2, SLAB).transpose(1, 2, 0, 3)
        scT = sc.reshape(128, 3, 2).transpose(1, 2, 0)[..., None] * (1.0 / 126.5)
        np.multiply(a8, scT, out=full[:, :, :, i, :])
    return full.reshape(3, C, HW), ok


def _upload_x1_chunked(rgb, chm, R):
    """Pack+upload the image slice one core-shard at a time so the first
    H2D bytes are in flight ~1.5ms in, instead of after the full pack."""
    jax = R['jax']
    devices = R['sharding'].mesh.devices.tolist()
    r4 = rgb.reshape(2, 128, N_CORES, SLAB)
    c4 = chm.reshape(2, 128, N_CORES, SLAB)
    shards, parts = [], []
    for r in range(N_CORES):
        xr = np.empty((128, 2, 2, SLAB), NP_F16)
        xr[:, 0] = r4[:, :, r, :].transpose(1, 0, 2)
        xr[:, 1] = c4[:, :, r, :].transpose(1, 0, 2)
        xr = xr.reshape(128, XS_W)
        parts.append(xr)
        shards.append(jax.device_put(xr, devices[r]))
    ga = jax.make_array_from_single_device_arrays(
        (N_CORES * 128, XS_W), R['sharding'], shards)
    return ga, parts


def kernel(**inputs):
    npi = {k: np.asarray(v) for k, v in inputs.items()}
    R = get_kernel()
    jax = R['jax']
    rgb, chm = npi.pop('rgb'), npi.pop('chm')
    d1, x1_parts = _upload_x1_chunked(rgb, chm, R)
    x2 = pack_x2(**npi)
    d2 = jax.device_put(x2, R['sharding'])
    try:
        outs = R['fn'](d1, d2)
        full, ok = _fetch_dequant(outs[0])
    except Exception:
        ok = False
    if not ok:
        res = run_packed(np.concatenate(x1_parts, axis=0), x2)
        return unpack_outputs(res)
    return tuple(full[i].reshape(1, C, H, W) for i in range(3))


if __name__ == '__main__':
    get_kernel()
    print('kernel built ok')


# revision 40
# speedup vs baseline: 1.1427x; 1.1427x over previous
"""Trainium2 Bass kernel for the cross-attention fusion module (nn_CAF), v2.

Strategy (8 NeuronCores, sequence-parallel):
  - Each core owns 800 query tokens. It computes softmax rows for its
    queries against all keys, accumulates a partial V @ A product, and the
    partials are summed with an on-chip ReduceScatter so core r ends up
    with output tokens [800r, 800r+800).
  - Host->device traffic is minimized (the axon tunnel is ~60-200MB/s):
    ONE packed fp16 input tensor per core holding the core's token slice,
    6 halo image rows for the depthwise conv, per-core masks, and a 1/8
    shard of the shared weight block. The weight block and the k tensor
    (computed per-slice, incl. pos-enc + bias) are AllGathered on-chip.
    Output is ONE packed fp16 [128, 4800] tensor per core.
  - The compiled XLA/NEFF executable is built once and cached; per call we
    only pack, dispatch, and unpack.
"""
import sys
sys.path.insert(0, '/opt/trn_rl_repo')
import os
import numpy as np

import concourse.bass as bass
import concourse.bacc as bacc
import concourse.tile as tile
from concourse import mybir

F32 = mybir.dt.float32
BF16 = mybir.dt.float16  # fp16
NP_F16 = np.float16

C = 256
RED = 32
H = W = 80
HW = H * W              # 6400
SCALE = RED ** -0.5
N_CORES = 8
SLAB = HW // N_CORES    # 800
ROWS = SLAB // W        # 10
EXP_BIAS = -3.0

AF = mybir.ActivationFunctionType
ALU = mybir.AluOpType

N_CHUNKS = [(i * 128, min(128, SLAB - i * 128)) for i in range((SLAB + 127) // 128)]
CHUNK_PAIRS = [(0, 1), (2, 3), (4, 5), (6,)]

INT8_OUT = True                       # ship outputs as int8 + per-row scales

# ---------------- packed input layout (all fp16) ----------------
# xin1: the big image slice, uploaded while the host packs xin2
XS_W = 4 * SLAB                       # core's tokens [img, cc, 800]
# xin2: per-core small data + W shard
AC2, AC_W = 0, ROWS                   # acore [128, 10]
H62 = 10                              # halo-row-valid mask, [1,512] row-major
OHP2 = 14                             # one-hot of core r-1 (8 cols)
OHN2 = 22                             # one-hot of core r+1 (8 cols)
WSH2 = 30                             # this core's shard of the W block
# --- shared W block (gathered on-chip), columns within [0, WCOL) ---
PEF_O, PEF_W = 0, 352                 # peflat [32,1408] as 4 row-blocks
VW_O, VW_W = 352, 512                 # v_w.T   [128, 2, 256]
PW_O, PW_W = 864, 1024                # proj.T  [128, 4, 256]
GW_O, GW_W = 1888, 1024               # gate.T  [128, 4, 256]
KW_O, KW_W = 2912, 64                 # k_w.T   [128, 2, 32]
QW_O, QW_W = 2976, 64                 # q_w.T   [128, 2, 32]
ABM_O, ABM_W = 3040, 80               # resize A.T tiled 4x [128, 80]
I32_O, I32_W = 3120, 32               # identity [32, 32]
DWW_O, DWW_W = 3152, 196              # dw weights [128, 4, 49]
PEB_O, PEB_W = 3348, 4                # dw bias    [128, 4]
KB_O, QB_O, VB_O, PB_O, GB_O = 3352, 3353, 3354, 3356, 3360
WCOL = 3368
WS = WCOL // N_CORES                  # 421
SCOL = WSH2 + WS                      # 451
OCOL = 3 * 2 * SLAB                   # 4800: [out|rgb|chm] x [cc] x [m]


def _mt(n, width=512):
    return [(i * width, min(width, n - i * width)) for i in range((n + width - 1) // width)]


def build_module():
    nc = bacc.Bacc('TRN2', target_bir_lowering=False, debug=False,
                   num_devices=N_CORES)

    I8 = mybir.dt.int8
    xin1 = nc.dram_tensor('xin1', [128, XS_W], BF16, kind='ExternalInput').ap()
    xin2 = nc.dram_tensor('xin2', [128, SCOL], BF16, kind='ExternalInput').ap()
    if INT8_OUT:
        # +32 int8 cols hold the 8 f32 per-row scales, bitcast to bytes
        oall = nc.dram_tensor('oall', [128, OCOL + 32], I8,
                              kind='ExternalOutput').ap()
    else:
        oall = nc.dram_tensor('oall', [128, OCOL], BF16, kind='ExternalOutput').ap()

    DBG = bool(os.environ.get('KERNEL_DEBUG'))
    if DBG:
        d_peq = nc.dram_tensor('d_peq', [32, SLAB], BF16, kind='ExternalOutput').ap()
        d_krep = nc.dram_tensor('d_krep', [128, 2, HW], BF16, kind='ExternalOutput').ap()
        d_qrep = nc.dram_tensor('d_qrep', [128, 2, SLAB], BF16, kind='ExternalOutput').ap()
        d_vt = nc.dram_tensor('d_vt', [128, 2, 7, C], BF16, kind='ExternalOutput').ap()
        d_vpad = nc.dram_tensor('d_vpad', [4, 128, 16 * 88], BF16, kind='ExternalOutput').ap()
        d_pp = nc.dram_tensor('d_pp', [4, 128, SLAB], BF16, kind='ExternalOutput').ap()
        d_rs = nc.dram_tensor('d_rs', [2, C, SLAB], F32, kind='ExternalOutput').ap()
        d_w = nc.dram_tensor('d_w', [128, WCOL], BF16, kind='ExternalOutput').ap()
        d_h6 = nc.dram_tensor('d_h6', [1, 512], BF16, kind='ExternalOutput').ap()

    groups = [list(range(N_CORES))]

    with tile.TileContext(nc) as tc:
        with (
            tc.tile_pool(name='persist', bufs=1) as PP,
            tc.tile_pool(name='dram', bufs=1, space='DRAM') as DR,
        ):
            k_rep = PP.tile([128, 2, HW], BF16, tag='k_rep')
            q_rep = PP.tile([128, 2, SLAB], BF16, tag='q_rep')
            vt_sb = PP.tile([128, 2, 7, C], BF16, tag='vt_sb')
            vpad = [PP.tile([128, 16 * 88], BF16, tag=f'vpad{i}', name=f'vpad{i}') for i in range(4)]
            vpodd = [PP.tile([128, 16 * 88], BF16, tag=f'vpodd{i}', name=f'vpodd{i}') for i in range(4)]
            pp_t = [PP.tile([128, SLAB], BF16, tag=f'pp{i}', name=f'pp{i}') for i in range(4)]
            dww_f = PP.tile([128, DWW_W], F32, tag='dww')
            dww_sb = dww_f[:].rearrange('p (t k) -> p t k', t=4)
            peb_sb = PP.tile([128, 4], F32, tag='peb')
            ones = PP.tile([1, HW], BF16, tag='ones')
            ebias = PP.tile([128, 1], F32, tag='ebias')
            peq_bf = PP.tile([32, SLAB], BF16, tag='peq_bf')

            wc_dr = DR.tile([128, WS], BF16, name='wc_dr')
            wall = DR.tile([N_CORES, 128, WS], BF16, name='wall')
            estrip = DR.tile([128, 2, 2, 480], BF16, name='estrip')
            gstr = DR.tile([N_CORES, 128, 2, 2, 480], BF16, name='gstr')
            kc_dr = DR.tile([32, 2, SLAB], BF16, name='kc_dr')
            kall = DR.tile([N_CORES, 32, 2, SLAB], BF16, name='kall')
            partial = [DR.tile([N_CORES, C, SLAB], F32, name=f'partial{i}') for i in range(2)]
            rs_out = [DR.tile([C, SLAB], F32, name=f'rs_out{i}') for i in range(2)]

            def w_unpack(dst, off, width, p0=0, p1=128):
                # dst: flat-col AP view [p1-p0, width]; src: W cols [off, off+width)
                b = off + width
                for r in range(off // WS, (b - 1) // WS + 1):
                    s0, s1 = max(off, r * WS), min(b, (r + 1) * WS)
                    nc.sync.dma_start(dst[:, s0 - off:s1 - off],
                                      wall[r, p0:p1, s0 - r * WS:s1 - r * WS])

            nc.vector.memset(ones[:], 1.0)
            nc.vector.memset(ebias[:], EXP_BIAS)

            # ---------------- phase A: convs + pos-enc resize ----------------
            with (
                tc.tile_pool(name='phA', bufs=1) as PA,
                tc.tile_pool(name='evA', bufs=4) as EA,
                tc.tile_pool(name='psA', bufs=2, space='PSUM') as PSA,
            ):
                xs_sb = PA.tile([128, XS_W], BF16, tag='xs_sb')
                s2_sb = PA.tile([128, SCOL], BF16, tag='s2_sb')
                # ---- load packed inputs; gather the shared W block ----
                nc.sync.dma_start(s2_sb[:], xin2[:])
                nc.sync.dma_start(xs_sb[:], xin1[:])
                nc.sync.dma_start(wc_dr[:], s2_sb[:, WSH2:WSH2 + WS])
                nc.gpsimd.collective_compute(
                    'AllGather', ALU.bypass, replica_groups=groups,
                    ins=[wc_dr[:].opt()], outs=[wall[:].opt()])
                dww_h = PA.tile([128, DWW_W + PEB_W], BF16, tag='dww_h')
                w_unpack(dww_h[:, 0:DWW_W], DWW_O, DWW_W)
                w_unpack(dww_h[:, DWW_W:], PEB_O, PEB_W)
                nc.vector.tensor_copy(dww_f[:], dww_h[:, 0:DWW_W])
                nc.vector.tensor_copy(peb_sb[:], dww_h[:, DWW_W:])

                xs_v = xs_sb[:].rearrange('p (i c m) -> p i c m', i=2, c=2)
                ac_v = s2_sb[:, AC2:AC2 + AC_W]

                # edge strips (first/last 3 image rows of the slice) -> AllGather
                for img in range(2):
                    for cc in range(2):
                        nc.sync.dma_start(estrip[:, img, cc, 0:240],
                                          xs_v[:, img, cc, 0:240])
                        nc.sync.dma_start(estrip[:, img, cc, 240:480],
                                          xs_v[:, img, cc, 560:800])
                nc.gpsimd.collective_compute(
                    'AllGather', ALU.bypass, replica_groups=groups,
                    ins=[estrip[:].opt()], outs=[gstr[:].opt()])
                gst_sb = PA.tile([128, N_CORES, 2, 2, 480], BF16, tag='gst')
                for s in range(N_CORES):
                    nc.sync.dma_start(gst_sb[:, s, :, :, :], gstr[s, :, :, :, :])
                # one-hot select: above rows = core r-1's bottom strip, below
                # rows = core r+1's top strip (zero at the image boundary).
                hal = PA.tile([128, 2, 2, 480], BF16, tag='hal')
                oh_f = PA.tile([128, 16], F32, tag='oh_f')
                nc.vector.tensor_copy(oh_f[:], s2_sb[:, OHP2:OHP2 + 16])
                for s in range(N_CORES):
                    ohp = oh_f[:, s:s + 1]
                    ohn = oh_f[:, 8 + s:8 + s + 1]
                    if s == 0:
                        nc.vector.tensor_scalar_mul(
                            hal[:, :, :, 0:240], gst_sb[:, s, :, :, 240:480], ohp)
                        nc.vector.tensor_scalar_mul(
                            hal[:, :, :, 240:480], gst_sb[:, s, :, :, 0:240], ohn)
                    else:
                        nc.vector.scalar_tensor_tensor(
                            hal[:, :, :, 0:240], gst_sb[:, s, :, :, 240:480], ohp,
                            hal[:, :, :, 0:240], op0=ALU.mult, op1=ALU.add)
                        nc.vector.scalar_tensor_tensor(
                            hal[:, :, :, 240:480], gst_sb[:, s, :, :, 0:240], ohn,
                            hal[:, :, :, 240:480], op0=ALU.mult, op1=ALU.add)
                xh_v = hal[:]

                pef_sb = PA.tile([32, 11 * 128], BF16, tag='pef')
                kw_f = PA.tile([128, KW_W], BF16, tag='kw')
                qw_f = PA.tile([128, QW_W], BF16, tag='qw')
                vw_f = PA.tile([128, VW_W], BF16, tag='vw')
                ab_sb = PA.tile([128, ABM_W], BF16, tag='ab')
                i32_sb = PA.tile([32, I32_W], BF16, tag='i32')
                kb_sb = PA.tile([1, RED], BF16, tag='kb')
                qb_sb = PA.tile([1, RED], BF16, tag='qb')
                vb_sb = PA.tile([1, C], BF16, tag='vb')
                h6_sb = PA.tile([1, 512], BF16, tag='h6')
                k_sl = PA.tile([32, 2, SLAB], BF16, tag='k_sl')

                for k4 in range(4):
                    nc.sync.dma_start(pef_sb[0:32, 352 * k4:352 * (k4 + 1)],
                                      wall[0, 32 * k4:32 * (k4 + 1), PEF_O:PEF_O + 352])
                w_unpack(kw_f[:], KW_O, KW_W)
                w_unpack(qw_f[:], QW_O, QW_W)
                w_unpack(vw_f[:], VW_O, VW_W)
                w_unpack(ab_sb[:], ABM_O, ABM_W)
                w_unpack(i32_sb[:], I32_O, I32_W, p0=0, p1=32)
                nc.sync.dma_start(kb_sb[:], wall[KB_O // WS, 0:32, KB_O % WS:KB_O % WS + 1])
                nc.sync.dma_start(qb_sb[:], wall[QB_O // WS, 0:32, QB_O % WS:QB_O % WS + 1])
                nc.sync.dma_start(vb_sb[:], wall[VB_O // WS, :, VB_O % WS:VB_O % WS + 2])
                nc.sync.dma_start(h6_sb[:], s2_sb[:, H62:H62 + 4])
                kw_v = kw_f[:].rearrange('p (c r) -> p c r', c=2)
                qw_v = qw_f[:].rearrange('p (c r) -> p c r', c=2)
                vw_v = vw_f[:].rearrange('p (c r) -> p c r', c=2)

                # --- pos-enc bilinear resize, stage 1 (contraction over i) ---
                t1f = PA.tile([128, 11, 80], BF16, tag='t1f')
                for ch in range(11):
                    ps = PSA.tile([128, 80], F32, tag='pa')
                    nc.tensor.matmul(ps[:], pef_sb[:, 128 * ch:128 * (ch + 1)],
                                     ab_sb[0:32, :], start=True, stop=True)
                    nc.vector.tensor_copy(t1f[:, ch, :], ps[:])
                # --- stage 2, core's rows only: peq[r, row*80+x] ---
                for r in range(RED):
                    ch, sub = r // 3, (r % 3) * 32
                    ps2 = PSA.tile([ROWS, 80], F32, tag='pa')
                    nc.tensor.matmul(ps2[:], ac_v[sub:sub + 32, :],
                                     t1f[sub:sub + 32, ch, :], start=True, stop=True)
                    ev2 = EA.tile([ROWS, 80], BF16, tag='pe2se')
                    nc.scalar.copy(ev2[:], ps2[:])
                    nc.sync.dma_start(peq_bf[r:r + 1, :], ev2[:])

                # --- k + q convs on the core's slice, + pe + bias ---
                for img in range(2):
                    for wv, bv, dst in ((kw_v, kb_sb, None), (qw_v, qb_sb, q_rep)):
                        ps = PSA.tile([32, SLAB], F32, tag='pa')
                        for m0, mw in _mt(SLAB):
                            nc.tensor.matmul(ps[:, m0:m0 + mw], wv[:, 0, :],
                                             xs_v[:, img, 0, m0:m0 + mw],
                                             start=True, stop=False)
                            nc.tensor.matmul(ps[:, m0:m0 + mw], wv[:, 1, :],
                                             xs_v[:, img, 1, m0:m0 + mw],
                                             start=False, stop=False)
                            nc.tensor.matmul(ps[:, m0:m0 + mw], bv[:],
                                             ones[:, m0:m0 + mw],
                                             start=False, stop=False)
                            nc.tensor.matmul(ps[:, m0:m0 + mw], i32_sb[:],
                                             peq_bf[:, m0:m0 + mw],
                                             start=False, stop=True)
                        if dst is None:
                            nc.scalar.copy(k_sl[0:32, img, :], ps[:])
                        else:
                            nc.scalar.copy(dst[0:32, img, :], ps[:])
                            nc.sync.dma_start(dst[32:64, img, :], dst[0:32, img, :])

                # --- gather k across cores -> full k_rep ---
                nc.sync.dma_start(kc_dr[:], k_sl[:])
                nc.gpsimd.collective_compute(
                    'AllGather', ALU.bypass, replica_groups=groups,
                    ins=[kc_dr[:].opt()], outs=[kall[:].opt()])
                for img in range(2):
                    for s in range(N_CORES):
                        nc.sync.dma_start(k_rep[0:32, img, SLAB * s:SLAB * (s + 1)],
                                          kall[s, :, img, :])
                    nc.sync.dma_start(k_rep[32:64, img, :], k_rep[0:32, img, :])

                # --- VT conv: vt[n, c] for core's n slice ---
                for img in range(2):
                    for ci, (n0, nw) in enumerate(N_CHUNKS):
                        ps = PSA.tile([128, C], F32, tag='pa')
                        nc.tensor.matmul(ps[:nw, :], xs_v[:, img, 0, n0:n0 + nw],
                                         vw_v[:, 0, :], start=True, stop=False)
                        nc.tensor.matmul(ps[:nw, :], xs_v[:, img, 1, n0:n0 + nw],
                                         vw_v[:, 1, :], start=False, stop=False)
                        nc.tensor.matmul(ps[:nw, :], ones[0:1, n0:n0 + nw],
                                         vb_sb[:], start=False, stop=True)
                        nc.vector.tensor_copy(vt_sb[:nw, img, ci, :], ps[:nw, :])

                # --- v conv on own rows + halo rows -> padded dwconv input ---
                for img in range(2):
                    for cc in range(2):
                        ps10 = PSA.tile([128, SLAB], F32, tag='pa')
                        for m0, mw in _mt(SLAB):
                            nc.tensor.matmul(ps10[:, m0:m0 + mw],
                                             vw_v[:, 0, 128 * cc:128 * (cc + 1)],
                                             xs_v[:, img, 0, m0:m0 + mw],
                                             start=True, stop=False)
                            nc.tensor.matmul(ps10[:, m0:m0 + mw],
                                             vw_v[:, 1, 128 * cc:128 * (cc + 1)],
                                             xs_v[:, img, 1, m0:m0 + mw],
                                             start=False, stop=False)
                            nc.tensor.matmul(ps10[:, m0:m0 + mw],
                                             vb_sb[:, 128 * cc:128 * (cc + 1)],
                                             ones[:, m0:m0 + mw],
                                             start=False, stop=True)
                        ps6 = PSA.tile([128, 480], F32, tag='pa6')
                        nc.tensor.matmul(ps6[:], vw_v[:, 0, 128 * cc:128 * (cc + 1)],
                                         xh_v[:, img, 0, :], start=True, stop=False)
                        nc.tensor.matmul(ps6[:], vw_v[:, 1, 128 * cc:128 * (cc + 1)],
                                         xh_v[:, img, 1, :], start=False, stop=False)
                        nc.tensor.matmul(ps6[:], vb_sb[:, 128 * cc:128 * (cc + 1)],
                                         h6_sb[:, 0:480], start=False, stop=True)
                        vp = vpad[img * 2 + cc]
                        nc.vector.memset(vp[:], 0.0)
                        vp3 = vp[:].rearrange('p (r x) -> p r x', r=16)
                        nc.vector.tensor_copy(
                            vp3[:, 0:3, 3:83],
                            ps6[:, 0:240].rearrange('p (r x) -> p r x', r=3))
                        nc.vector.tensor_copy(
                            vp3[:, 3:13, 3:83],
                            ps10[:].rearrange('p (r x) -> p r x', r=ROWS))
                        nc.vector.tensor_copy(
                            vp3[:, 13:16, 3:83],
                            ps6[:, 240:480].rearrange('p (r x) -> p r x', r=3))
                        vo = vpodd[img * 2 + cc]
                        nc.vector.tensor_copy(vo[:, 0:1407], vp[:, 1:1408])
                        nc.vector.memset(vo[:, 1407:1408], 0.0)

            # ------------- dwconv emission helper (interleaved later) -------------
            dw_items = []
            for t in range(4):
                for dy in range(7):
                    for dx in range(7):
                        dw_items.append((t, dy, dx))

            def emit_dw(n):
                for _ in range(n):
                    if not dw_items:
                        return
                    t, dy, dx = dw_items.pop(0)
                    par = dx % 2
                    base = vpodd[t] if par else vpad[t]
                    c0 = dx - par
                    src = base[:].rearrange('p (r x) -> p r x', x=88)[:, dy:dy + ROWS, c0:c0 + 80]
                    dst = pp_t[t][:].rearrange('p (r x) -> p r x', x=80)
                    wap = dww_sb[:, t, dy * 7 + dx:dy * 7 + dx + 1]
                    if dy == 0 and dx == 0:
                        nc.vector.tensor_scalar_mul(dst[:], src, wap)
                    else:
                        nc.vector.scalar_tensor_tensor(
                            dst[:], src, wap, dst[:], op0=ALU.mult, op1=ALU.add)

            # ---------------- attention phases ----------------
            with (
                tc.tile_pool(name='attn', bufs=1) as AT,
                tc.tile_pool(name='evT', bufs=4) as ET,
                tc.tile_pool(name='psS', bufs=1, space='PSUM') as PSS,
                tc.tile_pool(name='psV', bufs=3, space='PSUM') as PSV,
            ):
                HM = HW // 2
                racc = [AT.tile([128, 7, 8], F32, tag=f'racc{a}',
                                name=f'racc{a}') for a in range(2)]
                rsum = [AT.tile([128, 7], F32, tag=f'rsum{a}', name=f'rsum{a}') for a in range(2)]
                rinv = [AT.tile([128, 7], F32, tag=f'rinv{a}', name=f'rinv{a}') for a in range(2)]
                vtp = [AT.tile([128, 7, C], BF16, tag=f'vtp{a}', name=f'vtp{a}') for a in range(2)]
                H_MACROS = [(0, 1024), (1024, 1024), (2048, 1024), (3072, 128)]

                def s_exp_half(a, h, e_h):
                    qi, ki = (0, 1) if a == 0 else (1, 0)
                    for pair in CHUNK_PAIRS:
                        for mi, (m0, mw) in enumerate(H_MACROS):
                            pss = []
                            for g, ci in enumerate(pair):
                                n0, nw = N_CHUNKS[ci]
                                ps = PSS.tile([128, 1024], F32, tag='s')
                                pss.append((ps, ci, nw))
                                for sm0, smw in _mt(mw):
                                    km = h * HM + m0 + sm0
                                    nc.tensor.matmul(
                                        ps[:nw, sm0:sm0 + smw],
                                        q_rep[32 * g:32 * g + 32, qi, n0:n0 + nw],
                                        k_rep[32 * g:32 * g + 32, ki, km:km + smw],
                                        start=True, stop=True,
                                        tile_position=(32 * g, 0))
                            for ps, ci, nw in pss:
                                nc.scalar.activation(
                                    e_h[:nw, ci, m0:m0 + mw], ps[:nw, :mw],
                                    AF.Exp, bias=ebias[:nw, 0:1], scale=SCALE,
                                    accum_out=racc[a][:nw, ci, h * 4 + mi:h * 4 + mi + 1])

                def finalize(a):
                    nc.vector.reduce_sum(rsum[a][:], racc[a][:],
                                         axis=mybir.AxisListType.X)
                    nc.vector.reciprocal(rinv[a][:], rsum[a][:])
                    for ci, (n0, nw) in enumerate(N_CHUNKS):
                        nc.vector.tensor_scalar_mul(
                            vtp[a][:nw, ci, :], vt_sb[:nw, a, ci, :],
                            rinv[a][:nw, ci:ci + 1])

                def ve_half(a, h, e_h):
                    slabs = [h * 4 + i for i in range(4)]
                    for gi0 in range(0, 4, 2):
                        grp = slabs[gi0:gi0 + 2]
                        for cc in range(2):
                            pst = []
                            for slab in grp:
                                ps = PSV.tile([128, SLAB], F32, tag='ve')
                                pst.append(ps)
                                lm = (slab - h * 4) * SLAB
                                for ci, (n0, nw) in enumerate(N_CHUNKS):
                                    for off, mw in ((0, 512), (512, 288)):
                                        nc.tensor.matmul(
                                            ps[:, off:off + mw],
                                            vtp[a][:nw, ci, 128 * cc:128 * (cc + 1)],
                                            e_h[:nw, ci, lm + off:lm + off + mw],
                                            start=(ci == 0), stop=(ci == 6))
                            for k, slab in enumerate(grp):
                                ev = ET.tile([128, SLAB], F32, tag='vee')
                                nc.scalar.copy(ev[:], pst[k][:])
                                nc.sync.dma_start(
                                    partial[a][slab, 128 * cc:128 * (cc + 1), :],
                                    ev[:])
                            emit_dw(10)

                def e_tile(nm):
                    return AT.tile([128, 7, HM], BF16, tag='E', bufs=2, name=nm)

                e_a0 = e_tile('e_a0')
                s_exp_half(0, 0, e_a0)
                emit_dw(10)
                e_a1 = e_tile('e_a1')
                s_exp_half(0, 1, e_a1)
                finalize(0)
                emit_dw(10)
                ve_half(0, 0, e_a0)
                e_b0 = e_tile('e_b0')
                s_exp_half(1, 0, e_b0)
                ve_half(0, 1, e_a1)
                nc.gpsimd.collective_compute(
                    'ReduceScatter', ALU.add, replica_groups=groups,
                    ins=[partial[0].opt()], outs=[rs_out[0].opt()])
                e_b1 = e_tile('e_b1')
                s_exp_half(1, 1, e_b1)
                finalize(1)
                ve_half(1, 0, e_b0)
                ve_half(1, 1, e_b1)
                nc.gpsimd.collective_compute(
                    'ReduceScatter', ALU.add, replica_groups=groups,
                    ins=[partial[1].opt()], outs=[rs_out[1].opt()])
                emit_dw(200)
                if DBG:
                    nc.sync.dma_start(d_peq[:], peq_bf[:])
                    nc.sync.dma_start(d_krep[:], k_rep[:])
                    nc.sync.dma_start(d_qrep[:], q_rep[:])
                    nc.sync.dma_start(d_vt[:], vt_sb[:])
                    nc.sync.dma_start(d_h6[:], h6_sb[:])
                    for a in range(2):
                        nc.sync.dma_start(d_rs[a, :, :], rs_out[a][:])
                    for t in range(4):
                        nc.sync.dma_start(d_pp[t, :, :], pp_t[t][:])
                        nc.sync.dma_start(d_vpad[t, :, :], vpad[t][:])
                    for r in range(N_CORES):
                        wt = AT.tile([128, WS], BF16, tag='wtmp', bufs=2,
                                     name=f'wt{r}')
                        nc.sync.dma_start(wt[:], wall[r, :, :])
                        nc.sync.dma_start(d_w[:, WS * r:WS * (r + 1)], wt[:])

            # ---------------- phase D: dw-bias + proj + gate + blend ----------------
            with (
                tc.tile_pool(name='phD', bufs=1) as PD,
                tc.tile_pool(name='evD', bufs=4) as ED,
                tc.tile_pool(name='psD', bufs=2, space='PSUM') as PSD,
            ):
                pw_f = PD.tile([128, PW_W], BF16, tag='pw')
                gw_f = PD.tile([128, GW_W], BF16, tag='gw')
                pb_sb = PD.tile([1, 2 * C], BF16, tag='pb')
                gb_sb = PD.tile([1, C], BF16, tag='gb')
                w_unpack(pw_f[:], PW_O, PW_W)
                w_unpack(gw_f[:], GW_O, GW_W)
                nc.sync.dma_start(pb_sb[:], wall[PB_O // WS, :, PB_O % WS:PB_O % WS + 4])
                nc.sync.dma_start(gb_sb[:], wall[GB_O // WS, :, GB_O % WS:GB_O % WS + 2])
                pw_sb = pw_f[:].rearrange('p (t c) -> p t c', t=4)
                gw_sb = gw_f[:].rearrange('p (t c) -> p t c', t=4)
                pb_v = pb_sb[:].rearrange('p (i c) -> p i c', i=2)

                asum = PD.tile([128, 2, 2, SLAB], F32, tag='asum')
                for a in range(2):
                    for cc in range(2):
                        nc.sync.dma_start(asum[:, a, cc, :],
                                          rs_out[a][128 * cc:128 * (cc + 1), :])
                pi = PD.tile([128, 2, 2, SLAB], BF16, tag='pi')
                for img in range(2):
                    for cc in range(2):
                        t = img * 2 + cc
                        nc.vector.scalar_tensor_tensor(
                            pi[:, img, cc, :], pp_t[t][:], peb_sb[:, t:t + 1],
                            asum[:, img, cc, :], op0=ALU.add, op1=ALU.add)
                attf = PD.tile([128, 2, 2, SLAB], F32, tag='attf')
                gi_t = PD.tile([128, 4, SLAB], BF16, tag='gi')
                for img in range(2):
                    for mc in range(2):
                        ps = PSD.tile([128, SLAB], F32, tag='proj')
                        for m0, mw in _mt(SLAB):
                            nc.tensor.matmul(ps[:, m0:m0 + mw],
                                             pw_sb[:, img * 2, 128 * mc:128 * (mc + 1)],
                                             pi[:, img, 0, m0:m0 + mw],
                                             start=True, stop=False)
                            nc.tensor.matmul(ps[:, m0:m0 + mw],
                                             pw_sb[:, img * 2 + 1, 128 * mc:128 * (mc + 1)],
                                             pi[:, img, 1, m0:m0 + mw],
                                             start=False, stop=False)
                            nc.tensor.matmul(ps[:, m0:m0 + mw],
                                             pb_v[:, img, 128 * mc:128 * (mc + 1)],
                                             ones[:, m0:m0 + mw],
                                             start=False, stop=True)
                        nc.vector.tensor_copy(attf[:, img, mc, :], ps[:])
                        nc.scalar.copy(gi_t[:, img * 2 + mc, :], ps[:])
                gsig = PD.tile([128, 2, SLAB], BF16, tag='gsig')
                for mc in range(2):
                    ps = PSD.tile([128, SLAB], F32, tag='gate')
                    for m0, mw in _mt(SLAB):
                        for kc in range(4):
                            nc.tensor.matmul(ps[:, m0:m0 + mw],
                                             gw_sb[:, kc, 128 * mc:128 * (mc + 1)],
                                             gi_t[:, kc, m0:m0 + mw],
                                             start=(kc == 0), stop=False)
                        nc.tensor.matmul(ps[:, m0:m0 + mw],
                                         gb_sb[:, 128 * mc:128 * (mc + 1)],
                                         ones[:, m0:m0 + mw],
                                         start=False, stop=True)
                    nc.scalar.activation(gsig[:, mc, :], ps[:], AF.Sigmoid)
                # blend: out = chm + g*(rgb - chm); pack [out|rgb|chm]
                if INT8_OUT:
                    osc_sb = PD.tile([128, 8], F32, tag='osc')
                    nc.vector.memset(osc_sb[:], 0.0)
                for mc in range(2):
                    d = ED.tile([128, SLAB], F32, tag='d')
                    nc.vector.tensor_sub(d[:], attf[:, 0, mc, :], attf[:, 1, mc, :])
                    nc.vector.tensor_mul(d[:], d[:], gsig[:, mc, :])
                    do = ED.tile([128, SLAB], F32, tag='dof')
                    nc.vector.tensor_add(do[:], d[:], attf[:, 1, mc, :])
                    srcs = [do[:], attf[:, 0, mc, :], attf[:, 1, mc, :]]
                    for oi, src in enumerate(srcs):
                        col = oi * 2 * SLAB + mc * SLAB
                        if INT8_OUT:
                            rmax = ED.tile([128, 1], F32, tag='rmax')
                            nc.vector.reduce_max(rmax[:], src,
                                                 axis=mybir.AxisListType.X,
                                                 apply_absolute_value=True)
                            nc.vector.tensor_scalar_max(rmax[:], rmax[:], 1e-30)
                            nc.scalar.copy(
                                osc_sb[:, oi * 2 + mc:oi * 2 + mc + 1], rmax[:])
                            sinv = ED.tile([128, 1], F32, tag='sinv')
                            nc.vector.reciprocal(sinv[:], rmax[:])
                            nc.vector.tensor_scalar_mul(sinv[:], sinv[:], 126.5)
                            q = ED.tile([128, SLAB], I8, tag='q')
                            nc.vector.tensor_scalar_mul(q[:], src, sinv[:, 0:1])
                            nc.sync.dma_start(oall[:, col:col + SLAB], q[:])
                        else:
                            o16 = ED.tile([128, SLAB], BF16, tag='do')
                            nc.vector.tensor_copy(o16[:], src)
                            nc.sync.dma_start(oall[:, col:col + SLAB], o16[:])
                if INT8_OUT:
                    # issue from the activation engine: the scale writes are
                    # activation-engine copies, so queue order guarantees they
                    # land before this DMA reads the tile through the bitcast.
                    nc.scalar.dma_start(oall[:, OCOL:OCOL + 32],
                                        osc_sb[:].bitcast(I8))

    nc.compile()
    from concourse.bass_interp import get_hw_module
    nc.m = get_hw_module(nc.m)
    return nc


def build_resize_matrix():
    scale = 32 / 80.0
    A = np.zeros((80, 32), np.float64)
    for y in range(80):
        src = (y + 0.5) * scale - 0.5
        for i in range(32):
            A[y, i] = max(0.0, 1.0 - abs(src - i))
        A[y] /= A[y].sum()
    return A.astype(np.float32)


def _pack_peflat(pos_enc):
    out = np.zeros((32, 11 * 128), np.float32)
    for r in range(RED):
        ch, t = r // 3, r % 3
        out[:, 128 * ch + 32 * t:128 * ch + 32 * t + 32] = pos_enc[0, r].T
    return out


def build_wblock(pos_enc, q_w, q_b, k_w, k_b, v_w, v_b,
                 rgb_pe_w, rgb_pe_b, chm_pe_w, chm_pe_b,
                 rgb_proj_w, rgb_proj_b, chm_proj_w, chm_proj_b,
                 gate_w, gate_b):
    Wb = np.zeros((128, WCOL), NP_F16)
    pef = _pack_peflat(pos_enc)
    for k4 in range(4):
        Wb[32 * k4:32 * (k4 + 1), PEF_O:PEF_O + 352] = pef[:, 352 * k4:352 * (k4 + 1)]
    Wb[:, VW_O:VW_O + VW_W] = v_w.T.reshape(2, 128, C).transpose(1, 0, 2).reshape(128, VW_W)
    Wb[:, PW_O:PW_O + PW_W] = (np.stack([rgb_proj_w.T, chm_proj_w.T])
                               .reshape(4, 128, C).transpose(1, 0, 2).reshape(128, PW_W))
    Wb[:, GW_O:GW_O + GW_W] = gate_w.T.reshape(4, 128, C).transpose(1, 0, 2).reshape(128, GW_W)
    Wb[:, KW_O:KW_O + KW_W] = k_w.T.reshape(2, 128, RED).transpose(1, 0, 2).reshape(128, KW_W)
    Wb[:, QW_O:QW_O + QW_W] = q_w.T.reshape(2, 128, RED).transpose(1, 0, 2).reshape(128, QW_W)
    Wb[:, ABM_O:ABM_O + ABM_W] = np.tile(build_resize_matrix().T, (4, 1))
    Wb[0:32, I32_O:I32_O + I32_W] = np.eye(32, dtype=np.float32)
    Wb[:, DWW_O:DWW_O + DWW_W] = (np.stack([rgb_pe_w.reshape(C, 49), chm_pe_w.reshape(C, 49)])
                                  .reshape(4, 128, 49).transpose(1, 0, 2).reshape(128, DWW_W))
    Wb[:, PEB_O:PEB_O + PEB_W] = np.stack([rgb_pe_b, chm_pe_b]).reshape(4, 128).T
    Wb[0:32, KB_O] = k_b
    Wb[0:32, QB_O] = q_b
    Wb[:, VB_O:VB_O + 2] = v_b.reshape(128, 2)
    Wb[:, PB_O:PB_O + 4] = np.concatenate([rgb_proj_b, chm_proj_b]).reshape(128, 4)
    Wb[:, GB_O:GB_O + 2] = gate_b.reshape(128, 2)
    return Wb


def _pool():
    if 'pool' not in _CACHE:
        from concurrent.futures import ThreadPoolExecutor
        _CACHE['pool'] = ThreadPoolExecutor(max_workers=8)
    return _CACHE['pool']


def pack_x1(rgb, chm):
    x1 = np.empty((N_CORES, 128, XS_W), NP_F16)
    v = x1.reshape(N_CORES, 128, 2, 2, SLAB)

    def job(img, im):
        # [cc*128+p, r*800+j] -> [r, p, cc, j], cast f32->f16 in the assign
        v[:, :, img] = im.reshape(2, 128, N_CORES, SLAB).transpose(2, 1, 0, 3)

    fs = [_pool().submit(job, img, im) for img, im in ((0, rgb), (1, chm))]
    [f.result() for f in fs]
    return x1.reshape(N_CORES * 128, XS_W)


def pack_x2(pos_enc, q_w, q_b, k_w, k_b, v_w, v_b,
            rgb_pe_w, rgb_pe_b, chm_pe_w, chm_pe_b,
            rgb_proj_w, rgb_proj_b, chm_proj_w, chm_proj_b,
            gate_w, gate_b):
    if 'x2_tmpl' not in _CACHE:
        tm = np.zeros((N_CORES, 128, SCOL), NP_F16)
        h6 = np.zeros((N_CORES, 512), NP_F16)
        for r in range(N_CORES):
            for k in range(3):
                if 0 <= r * ROWS - 3 + k < H:
                    h6[r, k * W:(k + 1) * W] = 1.0
                if 0 <= r * ROWS + ROWS + k < H:
                    h6[r, (3 + k) * W:(4 + k) * W] = 1.0
            if r > 0:
                tm[r, :, OHP2 + r - 1] = 1.0
            if r < N_CORES - 1:
                tm[r, :, OHN2 + r + 1] = 1.0
        A = build_resize_matrix()
        for r in range(N_CORES):
            tm[r, :, AC2:AC2 + AC_W] = np.tile(
                A.T[:, r * ROWS:(r + 1) * ROWS], (4, 1))
        tm[:, :, H62:H62 + 4] = h6.reshape(N_CORES, 128, 4)
        _CACHE['x2_tmpl'] = tm
    x2 = _CACHE['x2_tmpl'].copy()
    Wb = build_wblock(pos_enc, q_w, q_b, k_w, k_b, v_w, v_b,
                      rgb_pe_w, rgb_pe_b, chm_pe_w, chm_pe_b,
                      rgb_proj_w, rgb_proj_b, chm_proj_w, chm_proj_b,
                      gate_w, gate_b)
    x2[:, :, WSH2:] = Wb.reshape(128, N_CORES, WS).transpose(1, 0, 2)
    return x2.reshape(N_CORES * 128, SCOL)


def pack_inputs(rgb, chm, **weights):
    return pack_x1(rgb, chm), pack_x2(**weights)


_CACHE = {}


def _build_runner():
    import jax
    from jax.sharding import Mesh, PartitionSpec
    from jax.experimental.shard_map import shard_map
    from concourse.bass2jax import (_bass_exec_p, partition_id_tensor,
                                    install_neuronx_cc_hook)
    install_neuronx_cc_hook()
    nc = build_module()

    partition_name = nc.partition_id_tensor.name if nc.partition_id_tensor else None
    in_names, out_names, out_avals = [], [], []
    for alloc in nc.m.functions[0].allocations:
        if not isinstance(alloc, mybir.MemoryLocationSet):
            continue
        name = alloc.memorylocations[0].name
        if alloc.kind == 'ExternalInput':
            if name != partition_name:
                in_names.append(name)
        elif alloc.kind == 'ExternalOutput':
            out_names.append(name)
            out_avals.append(jax.core.ShapedArray(
                tuple(alloc.tensor_shape), mybir.dt.np(alloc.dtype)))
    in_names_all = list(in_names)
    if partition_name is not None:
        in_names_all.append(partition_name)

    def _body(*args):
        operands = list(args)
        if partition_name is not None:
            operands.append(partition_id_tensor())
        return tuple(_bass_exec_p.bind(
            *operands, out_avals=tuple(out_avals), in_names=tuple(in_names_all),
            out_names=tuple(out_names), lowering_input_output_aliases=(),
            sim_require_finite=True, sim_require_nnan=True, nc=nc))

    devices = jax.devices()[:N_CORES]
    mesh = Mesh(np.asarray(devices), ('core',))
    fn = jax.jit(shard_map(
        _body, mesh=mesh, in_specs=(PartitionSpec('core'),) * len(in_names),
        out_specs=(PartitionSpec('core'),) * len(out_names), check_rep=False))
    sharding = jax.sharding.NamedSharding(mesh, PartitionSpec('core'))
    return {'nc': nc, 'fn': fn, 'in_names': in_names,
            'out_names': out_names, 'sharding': sharding, 'jax': jax}


def get_kernel():
    if 'R' not in _CACHE:
        _CACHE['R'] = _build_runner()
    return _CACHE['R']


def run_packed(x1, x2):
    R = get_kernel()
    outs = R['fn'](x1, x2)
    if len(outs) == 1:
        return {R['out_names'][0]: np.asarray(outs[0])}
    futs = [_pool().submit(np.asarray, o) for o in outs]
    return {n: f.result() for n, f in zip(R['out_names'], futs)}


def unpack_outputs(res):
    if INT8_OUT:
        raw = res['oall']
        a8 = raw[:, :OCOL].reshape(N_CORES, 128, 3, 2, SLAB)
        a8 = a8.transpose(2, 3, 1, 0, 4)            # [o, mc, p, r, j] view
        sc = np.ascontiguousarray(raw[:, OCOL:]).view('<f4')
        sc = sc.reshape(N_CORES, 128, 8)[:, :, :6].reshape(N_CORES, 128, 3, 2)
        sc = sc.transpose(2, 3, 1, 0)[..., None] * (1.0 / 126.5)
        full = np.multiply(a8, sc, dtype=np.float32).reshape(3, C, HW)
    else:
        arr = res['oall'].reshape(N_CORES, 128, 3, 2, SLAB)
        full = arr.transpose(2, 3, 1, 0, 4).reshape(3, C, HW).astype(np.float32)
    return tuple(full[i].reshape(1, C, H, W) for i in range(3))


def _outputs_valid(res):
    if not INT8_OUT:
        return True
    raw = res['oall']
    sc = np.ascontiguousarray(raw[:, OCOL:]).view('<f4')[:, :6]
    if not (np.isfinite(sc).all() and (np.abs(sc) < 1e4).all()):
        return False
    # quant maps each row's absmax to ~126, so a healthy payload has
    # near-full-scale values in every core's slab
    return bool(np.abs(raw[:, :OCOL]).max() >= 64)


def _fetch_dequant(o):
    """Stage async D2H for every output shard, then fetch+dequantize them
    in parallel threads so host math overlaps the transfer tail. Returns
    (full[3, C, HW] f32, ok)."""
    full = np.empty((3, 2, 128, N_CORES, SLAB), np.float32)
    flags = [True] * N_CORES
    shards = sorted(o.addressable_shards, key=lambda s: s.index[0].start or 0)
    datas = [sh.data for sh in shards]
    for d in datas:
        try:
            d.copy_to_host_async()
        except Exception:
            pass

    def work(i, d):
        raw = np.asarray(d)                             # [128, OCOL+32] int8
        sc = np.ascontiguousarray(raw[:, OCOL:]).view('<f4')[:, :6]
        if not (np.isfinite(sc).all() and (np.abs(sc) < 1e4).all()
                and np.abs(raw[:, :OCOL]).max() >= 64):
            flags[i] = False
            return
        a8 = raw[:, :OCOL].reshape(128, 3, 2, SLAB).transpose(1, 2, 0, 3)
        scT = sc.reshape(128, 3, 2).transpose(1, 2, 0)[..., None] * (1.0 / 126.5)
        np.multiply(a8, scT, out=full[:, :, :, i, :])
    fs = [_pool().submit(work, i, d) for i, d in enumerate(datas)]
    [f.result() for f in fs]
    return full.reshape(3, C, HW), all(flags)


def _upload_x1_chunked(rgb, chm, R):
    """Pack+upload the image slice one core-shard at a time so the first
    H2D bytes are in flight ~1.5ms in, instead of after the full pack."""
    jax = R['jax']
    devices = R['sharding'].mesh.devices.tolist()
    r4 = rgb.reshape(2, 128, N_CORES, SLAB)
    c4 = chm.reshape(2, 128, N_CORES, SLAB)
    shards, parts = [], []
    for r in range(N_CORES):
        xr = np.empty((128, 2, 2, SLAB), NP_F16)
        xr[:, 0] = r4[:, :, r, :].transpose(1, 0, 2)
        xr[:, 1] = c4[:, :, r, :].transpose(1, 0, 2)
        xr = xr.reshape(128, XS_W)
        parts.append(xr)
        shards.append(jax.device_put(xr, devices[r]))
    ga = jax.make_array_from_single_device_arrays(
        (N_CORES * 128, XS_W), R['sharding'], shards)
    return ga, parts


def kernel(**inputs):
    npi = {k: np.asarray(v) for k, v in inputs.items()}
    R = get_kernel()
    jax = R['jax']
    rgb, chm = npi.pop('rgb'), npi.pop('chm')
    d1, x1_parts = _upload_x1_chunked(rgb, chm, R)
    x2 = pack_x2(**npi)
    d2 = jax.device_put(x2, R['sharding'])
    try:
        outs = R['fn'](d1, d2)
        full, ok = _fetch_dequant(outs[0])
    except Exception:
        ok = False
    if not ok:
        res = run_packed(np.concatenate(x1_parts, axis=0), x2)
        return unpack_outputs(res)
    return tuple(full[i].reshape(1, C, H, W) for i in range(3))


if __name__ == '__main__':
    get_kernel()
    print('kernel built ok')
